# revision 22
# baseline (speedup 1.0000x reference)
"""Multi-head attention (B=2, S=2048, D=1024, H=16) on 8 Trainium2 cores.

Sharding: data-parallel over batch (2) x tensor-parallel over heads (16 -> 4
per core). Core c handles batch c//4, heads 4*(c%4) .. 4*(c%4)+3. Each core
computes its heads' Q/K/V projections (column-sliced weights), flash-style
attention with transposed-score layout, and a partial output projection
(row-sliced Wo). Host sums the 4 bf16 partials per batch and adds bv@Wo+bo.

Design notes (evolved against perfetto/NTFF traces):
  - Everything the PE touches is bf16: x arrives host-transposed [D,S] and
    weights host-packed partition-major, so there are no PE transposes and
    every DMA run is 4KB-contiguous per partition. bf16 also halves SBUF
    read power: the core's power manager duty-cycles all engines to 50%
    when load is too dense (observed via HAM records), so energy is wall
    time; f32r matmuls throttled ~38% of the run, bf16 ~10%.
  - Projections pipeline with the input DMAs: the six earliest-needed
    K/Q accumulator groups run d-chunk-outer in PSUM banks borrowed from
    the attention-phase tag rings (a dedicated pool's exit barrier would
    stall the first scores on all eight bias drains), ordered so the two
    tiles the first scores need drain first. The V projection and the
    remaining K/Q groups ride inside the first attention block's j-loop.
  - Attention blocks (pair p, 512 queries) run a 16-step j-loop:
    exp(j) on ACT, scores(j+1) emitted before AV(j) so the PE FIFO never
    queues behind the activation, and the next block's first scores are
    emitted at j=15 so ACT never gaps at block boundaries. softmax
    denominators ride as a 65th row of the AV matmul (v augmented with a
    ones column).
  - Normalization is a DVE reciprocal (deferred into the next block's
    j-loop at j=13, off the fin-cast path) + rank-1 ones-outer-product
    broadcast on the PE + DVE multiply into the bf16 Wo lhsT, all one
    block late so the PE never waits on it.
  - Output projection tiles are spread one-per-j-step into later blocks
    (j>=4, after the block-boundary DVE drain chain clears); outputs stage
    to bf16 SBUF and DMA out per 128-row tile. The host gather upcasts.
  - The final block's drains/casts use the scalar engine (idle at the
    tail and its Copy shares the exp table, so no table swap); elsewhere
    copies stay on DVE because extra concurrent engines raise the power
    duty-cycle throttle more than they help.
"""

import numpy as np

B, S, D, H, DK = 2, 2048, 1024, 16, 64
HPC = 4          # heads per core
HD = HPC * DK    # 256 projected dims per core
P = 128
NB = 512
NCORES = 8

_CACHE = {}


def _install_tile_drain_fix():
    """TileContext._drain_and_barrier piles every outstanding sem wait onto
    one Drain instruction; this walrus build rejects >1 sync wait per
    instruction. Split the extra waits across single-wait NOPs."""
    import concourse.tile as tile
    from concourse.vector_clock import ScopedClock

    if getattr(tile.TileContext, "_ant_drain_fix", False):
        return

    def _drain_and_barrier_split(self, tick_clock, wait_clock):
        drain_inst = self.nc.sync.drain()
        wait_clock.add_sem_waits(
            drain_inst.ins, ScopedClock({None: tick_clock.global_clock})
        )
        waits = list(drain_inst.ins.sync_info.on_wait or [])
        if len(waits) > 1:
            drain_inst.ins.sync_info.on_wait = waits[:1]
            for w in waits[1:]:
                n = self.nc.sync.nop(nofuse=True)
                si = n.ins.sync_info
                if si is None:
                    import bass_rust

                    n.ins.sync_info = bass_rust.SyncInfo(on_wait=[w], on_update=[])
                else:
                    si.on_wait = [w]

        self.nc.all_engine_barrier()
        assert self.sems is not None
        popped = self.nc._tile_sem_poison_stack.pop()
        assert popped is self._sem_poison
        self.nc.clear_and_free_semaphores(list(self.sems.allocated().values()))
        self.nc.all_engine_barrier()

    tile.TileContext._drain_and_barrier = _drain_and_barrier_split
    tile.TileContext._ant_drain_fix = True


def _split_excess_waits(nc):
    """walrus's per-struct sync-wait capacity is small (observed: 1 for the
    self-loading-weight Matmult S3_LW struct, 2 for TPB_CTRL/Drain). Tile's
    wait assignment can leave many waits on one instruction; hoist the excess
    onto NOPs on the same engine immediately before it."""
    import concourse.mybir as mybir

    nid = [0]
    for f in nc.m.functions:
        for bb in f.blocks:
            out = []
            changed = False
            for inst in bb.instructions:
                si = getattr(inst, "sync_info", None)
                waits = list(si.on_wait) if si is not None and si.on_wait else []
                cap = 1
                if len(waits) > cap:
                    extra = waits[cap:]
                    for k in range(0, len(extra), 2):
                        nid[0] += 1
                        out.append(
                            mybir.InstEventSemaphore(
                                name=f"I-waitsplit-{nid[0]}",
                                ins=[],
                                outs=[],
                                sync_info=mybir.SyncInfo(
                                    on_wait=extra[k:k + 2], on_update=[]
                                ),
                                engine=inst.engine,
                            )
                        )
                    si.on_wait = waits[:cap]
                    changed = True
                out.append(inst)
            if changed:
                bb.instructions = out


def _recip_fast(nc, out, in_):
    with nc.allow_low_precision("fp22 recip feeds f32r matmul"):
        nc.vector.reciprocal(out=out, in_=in_)


def _build_program():
    import concourse.bass as bass
    import concourse.mybir as mybir
    from concourse.tile import TileContext

    _install_tile_drain_fix()

    f32 = mybir.dt.float32
    f32r = mybir.dt.float32r
    bf16 = mybir.dt.bfloat16
    Exp = mybir.ActivationFunctionType.Exp

    nc = bass.Bass()

    xt = nc.dram_tensor("xt", [D, S], bf16, kind="ExternalInput")
    # weights arrive pre-packed partition-major from the host, so each DMA
    # moves 4KB-contiguous runs per partition (descriptor-count-bound
    # otherwise: the transposed gather was ~4x slower)
    wq = nc.dram_tensor("wq", [P, (D // P) * HD], bf16, kind="ExternalInput")
    wk = nc.dram_tensor("wk", [P, (D // P) * HD], bf16, kind="ExternalInput")
    wv = nc.dram_tensor("wv", [P, (D // P) * HD], bf16, kind="ExternalInput")
    wo = nc.dram_tensor("wo", [P, 2 * D], bf16, kind="ExternalInput")
    bqt = nc.dram_tensor("bqt", [P, 2], f32, kind="ExternalInput")
    bkt = nc.dram_tensor("bkt", [P, 2], f32, kind="ExternalInput")
    outp = nc.dram_tensor("outp", [S, D], bf16, kind="ExternalOutput")

    NDC = D // P      # 8 d-chunks
    NST = S // P      # 16 sequence tiles
    NSB = S // NB     # 4 sequence blocks

    with TileContext(nc) as tc:
        with tc.tile_pool(name="consts", bufs=1) as consts:
            # memset on a float32r AP emits invalid ISA; write the f32 bit
            # pattern of 1.0 through a uint32 view instead
            onesg = consts.tile([33, DK], bf16)
            nc.vector.memset(onesg.bitcast(mybir.dt.uint16), 0x3F80)

            # DMA queue order follows first-use: biases and K/Q weights
            # (the wave-1 projection needs them first), then xT chunks (the
            # d-outer projection starts as soon as chunk 0 lands), V/O last
            bq_sb = consts.tile([P, 2], f32)
            nc.sync.dma_start(bq_sb[:], bqt[:])
            bk_sb = consts.tile([P, 2], f32)
            nc.sync.dma_start(bk_sb[:], bkt[:])
            wk_sb = consts.tile([P, NDC, HD], bf16)
            nc.sync.dma_start(wk_sb[:], wk.rearrange("p (c h) -> p c h", c=NDC))
            wq_sb = consts.tile([P, NDC, HD], bf16)
            nc.sync.dma_start(wq_sb[:], wq.rearrange("p (c h) -> p c h", c=NDC))
            xT = consts.tile([P, NDC, S], bf16)
            for d in range(NDC):
                nc.sync.dma_start(xT[:, d, :], xt[d * P:(d + 1) * P, :])
            wv_sb = consts.tile([P, NDC, HD], bf16)
            nc.sync.dma_start(wv_sb[:], wv.rearrange("p (c h) -> p c h", c=NDC))
            wo_sb = consts.tile([P, 2, D], bf16)
            nc.sync.dma_start(wo_sb[:], wo.rearrange("p (c d) -> p c d", c=2))

            with tc.tile_pool(name="acts", bufs=1) as acts:
                # pair-packed transposed projections: [2 heads x 64, S]
                qT = acts.tile([P, 2, S], bf16)
                kT = acts.tile([P, 2, S], bf16)
                # v augmented with a ones column (row 65 of the AV matmul
                # accumulates the softmax denominator): [s, j-tile, head, 65]
                va = acts.tile([P, NST, HPC, DK + 1], bf16)
                nc.vector.memset(va.bitcast(mybir.dt.uint16), 0x3F80)
                # Wo lhsT: [head-dim pair-chunk, pair, i]
                stack = acts.tile([P, 2, S], bf16)
                # staging tile for both heads' softmax denominators, at
                # partitions 0 and 32 so one reciprocal covers both; the
                # filler rows are preset to 1.0 so recip never sees junk
                sums_sb = acts.tile([33, NB], bf16)
                nc.vector.memset(sums_sb.bitcast(mybir.dt.uint16), 0x3F80)
                # tiny warm-up exp: hoists the 1.3us ACT table load out of
                # the first real activation's critical path
                warm = acts.tile([1, 2], bf16)
                nc.scalar.activation(warm[:], sums_sb[0:1, 0:2], Exp, scale=0.125)

                # ---- Attention + output projection. V projection rides in
                # block (0,0); Q projections for blocks 1-3 ride in blocks
                # (0,1), (1,0), (2,0). PSUM: sc 4 banks, av0/av1 2, aux 2.
                with (
                    tc.tile_pool(name="ph2", bufs=1) as ph2,
                    tc.tile_pool(name="ph2p", bufs=1, space="PSUM") as ph2p,
                ):
                    # ---- Wave-1 projection: K pair 0 (all blocks), Q block
                    # 0 pair 0, K block 0 pair 1 - six PSUM accumulator
                    # groups borrowed from phase-2's own tag rings (no
                    # separate pool: a pool exit would put an all-drain
                    # barrier in front of the first scores). d-chunk-outer:
                    # layer d only needs xT chunk d, so compute pipelines
                    # with the xT DMAs. The two tiles the first scores need
                    # (k sb0/p0, q b0/p0) drain first, and they sit in the
                    # same sc-ring slot the first scores will request.
                    w1sc0 = ph2p.tile([P, 2 * NB], f32, tag="sc", bufs=2,
                                      name="w1sc0")
                    w1a = ph2p.tile([P, NB], f32, tag="aux", bufs=2,
                                    name="w1a")
                    w1b = ph2p.tile([P, NB], f32, tag="aux", bufs=2,
                                    name="w1b")
                    w1sc1 = ph2p.tile([P, 2 * NB], f32, tag="sc", bufs=2,
                                      name="w1sc1")
                    wave1 = [
                        (w1sc0[:, 0:NB], "k", 0, 0),
                        (w1sc0[:, NB:2 * NB], "q", 0, 0),
                        (w1a[:], "k", 1, 0),
                        (w1b[:], "k", 2, 0),
                        (w1sc1[:, 0:NB], "k", 3, 0),
                        (w1sc1[:, NB:2 * NB], "k", 0, 1),
                    ]
                    for d in range(NDC):
                        for acc, kind, sb_, p_ in wave1:
                            w_sb = wk_sb if kind == "k" else wq_sb
                            nc.tensor.matmul(
                                acc,
                                w_sb[:, d, p_ * P:(p_ + 1) * P],
                                xT[:, d, sb_ * NB:(sb_ + 1) * NB],
                                start=(d == 0),
                                stop=(d == NDC - 1),
                            )
                            if d == NDC - 1:
                                dT, b_sb = (
                                    (kT, bk_sb) if kind == "k" else (qT, bq_sb)
                                )
                                nc.vector.tensor_scalar_add(
                                    out=dT[:, p_, sb_ * NB:(sb_ + 1) * NB],
                                    in0=acc,
                                    scalar1=b_sb[:, p_:p_ + 1],
                                )
                    def emit_vp(jt):
                        vp = ph2p.tile(
                            [P, NB], f32, tag="aux", bufs=2, name=f"vp{jt}"
                        )
                        for d in range(NDC):
                            nc.tensor.matmul(
                                vp[:, 0:HD],
                                xT[:, d, jt * P:(jt + 1) * P],
                                wv_sb[:, d, :],
                                start=(d == 0),
                                stop=(d == NDC - 1),
                            )
                        nc.vector.tensor_copy(
                            out=va[:, jt, :, 0:DK],
                            in_=vp[:, 0:HD].rearrange("p (h e) -> p h e", h=HPC),
                        )

                    def emit_burst(kind, sb_, p_):
                        # one full projection group (8 matmuls + bias drain)
                        # inside block (0,0)'s j-loop: finishes wave-1's
                        # leftovers (k sb3/p1, q b0/p1) without delaying the
                        # first exp
                        ps = ph2p.tile(
                            [P, NB], f32, tag="aux", bufs=2,
                            name=f"burst_{kind}{sb_}_{p_}",
                        )
                        w_sb = wk_sb if kind == "k" else wq_sb
                        for d in range(NDC):
                            nc.tensor.matmul(
                                ps[:],
                                w_sb[:, d, p_ * P:(p_ + 1) * P],
                                xT[:, d, sb_ * NB:(sb_ + 1) * NB],
                                start=(d == 0),
                                stop=(d == NDC - 1),
                            )
                        dT, b_sb = (kT, bk_sb) if kind == "k" else (qT, bq_sb)
                        nc.vector.tensor_scalar_add(
                            out=dT[:, p_, sb_ * NB:(sb_ + 1) * NB],
                            in0=ps[:],
                            scalar1=b_sb[:, p_:p_ + 1],
                        )

                    # qproj for block qb, split in 16 single-matmul steps
                    qstate = {}

                    def emit_qstep(qb, step):
                        p_, d = divmod(step, NDC)
                        if d == 0:
                            qstate[(qb, p_)] = ph2p.tile(
                                [P, NB], f32, tag="aux", bufs=2,
                                name=f"q{qb}_{p_}",
                            )
                        psq = qstate[(qb, p_)]
                        nc.tensor.matmul(
                            psq[:],
                            wq_sb[:, d, p_ * P:(p_ + 1) * P],
                            xT[:, d, qb * NB:(qb + 1) * NB],
                            start=(d == 0),
                            stop=(d == NDC - 1),
                        )
                        if d == NDC - 1:
                            nc.vector.tensor_scalar_add(
                                out=qT[:, p_, qb * NB:(qb + 1) * NB],
                                in0=psq[:],
                                scalar1=bq_sb[:, p_:p_ + 1],
                            )

                    def emit_scores(ib, p, j):
                        sc = ph2p.tile(
                            [P, 2 * NB], f32, tag="sc", bufs=2,
                            name=f"sc{ib}_{p}_{j}",
                        )
                        i0 = ib * NB
                        nc.tensor.matmul(
                            sc[:, 0:NB],
                            kT[0:DK, p, j * P:(j + 1) * P],
                            qT[0:DK, p, i0:i0 + NB],
                            tile_position=(0, 0),
                        )
                        nc.tensor.matmul(
                            sc[:, NB:2 * NB],
                            kT[DK:2 * DK, p, j * P:(j + 1) * P],
                            qT[DK:2 * DK, p, i0:i0 + NB],
                            tile_position=(64, 0),
                        )
                        return sc

                    def make_fin(it, tail=False):
                        # one output row-tile: both 512-halves of the final
                        # projection, staged to bf16 SBUF, then one DMA. The
                        # post-loop fins stage via the scalar engine (idle at
                        # the tail; its copy shares the exp table) so the
                        # drain chain doesn't serialize on DVE
                        def go():
                            ot = ph2.tile(
                                [P, D], bf16, tag="ot", bufs=2, name=f"ot{it}"
                            )
                            for nbi in range(2):
                                ps = ph2p.tile(
                                    [P, NB], f32, tag="aux", bufs=2,
                                    name=f"fin{it}_{nbi}",
                                )
                                for pch in range(2):
                                    nc.tensor.matmul(
                                        ps[:],
                                        stack[:, pch, it * P:(it + 1) * P],
                                        wo_sb[:, pch,
                                              nbi * NB:(nbi + 1) * NB],
                                        start=(pch == 0),
                                        stop=(pch == 1),
                                    )
                                dst = ot[:, nbi * NB:(nbi + 1) * NB]
                                if tail:
                                    nc.scalar.copy(out=dst, in_=ps[:])
                                else:
                                    nc.vector.tensor_copy(out=dst, in_=ps[:])
                            nc.sync.dma_start(outp[it * P:(it + 1) * P, :], ot[:])
                        return go

                    def finish_norm(ib, p, po_sbs, rc33, tail=False):
                        # broadcast each head's 1/sumexp across 64 partitions
                        # (rank-1 matmul) and scale the raw AV numerators into
                        # the Wo lhsT. Emitted one block late so the PE never
                        # waits on the DVE reciprocals. Returns the final
                        # projection closures (spread into a later j-loop).
                        i0 = ib * NB
                        for hh in range(2):
                            bc = ph2p.tile(
                                [P, NB], f32, tag="aux", bufs=2,
                                name=f"bc{ib}_{p}_{hh}",
                            )
                            nc.tensor.matmul(
                                bc[0:DK, :],
                                onesg[32 * hh:32 * hh + 1, :],
                                rc33[32 * hh:32 * hh + 1, :],
                                tile_position=(32 * hh, 0),
                            )
                            nc.vector.tensor_tensor(
                                out=stack[hh * DK:(hh + 1) * DK, p, i0:i0 + NB],
                                in0=po_sbs[hh][0:DK, :],
                                in1=bc[0:DK, :],
                                op=mybir.AluOpType.mult,
                            )
                        if p != 1:
                            return []
                        return [
                            make_fin(ib * (NB // P) + t, tail)
                            for t in range(NB // P)
                        ]

                    # extra projection work carried by each block's j-loop;
                    # fin batches ride two blocks after their AV block, in
                    # blocks with no projection job (aux-ring discipline:
                    # a held qproj slot must never interleave with fins)
                    carry = {(0, 0): "v", (0, 1): 1, (1, 0): 2, (2, 0): 3}
                    blocks = [(ib, p) for ib in range(NSB) for p in range(2)]

                    pending_norm = None
                    pending_fins = []
                    pending_recip = None
                    sc = None
                    for bi, (ib, p) in enumerate(blocks):
                        job = carry.get((ib, p))
                        last = bi == len(blocks) - 1
                        po0 = ph2p.tile(
                            [P, NB], f32, tag="av0", bufs=1,
                            name=f"po0_{ib}_{p}",
                        )
                        po1 = ph2p.tile(
                            [P, NB], f32, tag="av1", bufs=1,
                            name=f"po1_{ib}_{p}",
                        )
                        if sc is None:
                            sc = emit_scores(ib, p, 0)
                        if job == "v":
                            emit_vp(0)
                        fin_q = pending_fins
                        pending_fins = []
                        for j in range(NST):
                            ex = ph2.tile(
                                [P, 2 * NB], bf16, tag="ex", bufs=5,
                                name=f"ex{ib}_{p}_{j}",
                            )
                            nc.scalar.activation(
                                ex[:], sc[:], Exp, scale=0.125
                            )
                            # scores for the next step (or the next block's
                            # first step) are emitted before AV(j) so the PE
                            # FIFO never queues behind exp(j) and the ACT
                            # engine never gaps at block boundaries
                            if j + 1 < NST:
                                sc = emit_scores(ib, p, j + 1)
                            elif bi + 1 < len(blocks):
                                sc = emit_scores(*blocks[bi + 1], 0)
                            if job == "v":
                                # block (0,0) also finishes the projection
                                # groups wave-1 had no PSUM room for, as
                                # j-step bursts
                                if j < 4:
                                    emit_burst(*(
                                        ("k", 1, 1), ("q", 0, 1),
                                        ("k", 2, 1), ("k", 3, 1),
                                    )[j])
                                if j + 1 < NST:
                                    emit_vp(j + 1)
                            elif job is not None:
                                emit_qstep(job, j)
                            elif fin_q and j >= 4:
                                # j>=4: the block-boundary DVE chain (po
                                # drains, sums, norms) must clear before fin
                                # casts queue up, or fin matmuls block AV
                                fin_q.pop(0)()
                            # the previous block's reciprocal runs mid-loop so
                            # it never delays this block's fin casts on DVE
                            if pending_recip is not None and j == 13:
                                pending_recip()
                                pending_recip = None
                            nc.tensor.matmul(
                                po0[0:DK + 1, :],
                                va[:, j, 2 * p, :],
                                ex[:, 0:NB],
                                start=(j == 0),
                                stop=(j == NST - 1),
                            )
                            nc.tensor.matmul(
                                po1[0:DK + 1, :],
                                va[:, j, 2 * p + 1, :],
                                ex[:, NB:2 * NB],
                                start=(j == 0),
                                stop=(j == NST - 1),
                            )
                        assert not fin_q, (ib, p, len(fin_q))
                        # drain both accumulator banks so the next block's AV
                        # can start (all on DVE: the core power-throttles when
                        # too many engines run dense, so spreading copies onto
                        # ACT mid-stream backfires)
                        po_sbs = []
                        for hh, po in ((0, po0), (1, po1)):
                            po_sb = ph2.tile(
                                [DK, NB], bf16, tag="posb", bufs=4,
                                name=f"posb{ib}_{p}_{hh}",
                            )
                            if last:
                                # tail: scalar engine (idle there) drains the
                                # accumulators while DVE runs the reciprocal
                                nc.scalar.copy(out=po_sb[:], in_=po[0:DK, :])
                            else:
                                nc.vector.tensor_copy(
                                    out=po_sb[:], in_=po[0:DK, :]
                                )
                            po_sbs.append(po_sb)
                        # partition-shifting copies (64 -> 0/32): proven
                        # on DVE, keep them there
                        nc.vector.tensor_copy(
                            out=sums_sb[0:1, :], in_=po0[DK:DK + 1, :]
                        )
                        nc.vector.tensor_copy(
                            out=sums_sb[32:33, :], in_=po1[DK:DK + 1, :]
                        )
                        rc33 = ph2.tile(
                            [33, NB], bf16, tag="rc", bufs=2,
                            name=f"rc{ib}_{p}",
                        )
                        if last:
                            _recip_fast(nc, rc33[:], sums_sb[:])
                        else:
                            def _defer(rc=rc33):
                                _recip_fast(nc, rc[:], sums_sb[:])
                            pending_recip = _defer
                        # norms for the previous block go here (not at block
                        # start): their bc matmuls consume a reciprocal that
                        # by now is a full block old, so the PE never waits
                        if pending_norm is not None:
                            pending_fins = finish_norm(*pending_norm)
                        pending_norm = (ib, p, po_sbs, rc33)
                    for go in pending_fins + finish_norm(*pending_norm, tail=True):
                        go()

    _split_excess_waits(nc)
    return nc


def _get_program():
    if "nc" not in _CACHE:
        _CACHE["nc"] = _build_program()
    return _CACHE["nc"]


def kernel(x, Wq, bq, Wk, bk, Wv, bv, Wo, bo, _trace=False):
    import ml_dtypes
    from concourse.bass_utils import run_bass_kernel_spmd

    bf16 = ml_dtypes.bfloat16
    x = np.asarray(x, dtype=np.float32)
    Wq = np.asarray(Wq, dtype=np.float32)
    Wk = np.asarray(Wk, dtype=np.float32)
    Wv = np.asarray(Wv, dtype=np.float32)
    Wo = np.asarray(Wo, dtype=np.float32)
    bq = np.asarray(bq, dtype=np.float32)
    bk = np.asarray(bk, dtype=np.float32)
    bv = np.asarray(bv, dtype=np.float32)
    bo = np.asarray(bo, dtype=np.float32)

    def pack(w):
        # [NDC*P, F] -> partition-major [P, NDC*F] so SBUF DMA runs are
        # contiguous per partition
        ndc = w.shape[0] // P
        return np.ascontiguousarray(
            w.reshape(ndc, P, -1).transpose(1, 0, 2).reshape(P, -1)
        ).astype(bf16)

    xtb = [np.ascontiguousarray(x[b].T).astype(bf16) for b in range(B)]
    in_maps = []
    for c in range(NCORES):
        b = c // 4
        cs = (c % 4) * HD
        in_maps.append({
            "xt": xtb[b],
            "wq": pack(Wq[:, cs:cs + HD]),
            "wk": pack(Wk[:, cs:cs + HD]),
            "wv": pack(Wv[:, cs:cs + HD]),
            "wo": pack(Wo[cs:cs + HD, :]),
            "bqt": np.ascontiguousarray(bq[cs:cs + HD].reshape(2, P).T),
            "bkt": np.ascontiguousarray(bk[cs:cs + HD].reshape(2, P).T),
        })

    nc = _get_program()
    res = run_bass_kernel_spmd(
        nc, in_maps, core_ids=list(range(NCORES)), trace=_trace
    )

    cvec = (bv @ Wo + bo).astype(np.float32)
    out = np.empty((B, S, D), dtype=np.float32)
    for b in range(B):
        acc = res.results[4 * b]["outp"].astype(np.float64)
        for c in range(4 * b + 1, 4 * b + 4):
            acc = acc + res.results[c]["outp"]
        out[b] = (acc + cvec).astype(np.float32)

    if _trace:
        _CACHE["last_results"] = res
    return out


# revision 26
# speedup vs baseline: 1.0056x; 1.0056x over previous
"""Multi-head attention (B=2, S=2048, D=1024, H=16) on 8 Trainium2 cores.

Sharding: data-parallel over batch (2) x tensor-parallel over heads (16 -> 4
per core). Core c handles batch c//4, heads 4*(c%4) .. 4*(c%4)+3. Each core
computes its heads' Q/K/V projections (column-sliced weights), flash-style
attention with transposed-score layout, and a partial output projection
(row-sliced Wo). Host sums the 4 bf16 partials per batch and adds bv@Wo+bo.

Design notes (evolved against perfetto/NTFF traces):
  - Everything the PE touches is bf16: x arrives host-transposed [D,S] and
    weights host-packed partition-major, so there are no PE transposes and
    every DMA run is 4KB-contiguous per partition. bf16 also halves SBUF
    read power: the core's power manager duty-cycles all engines to 50%
    when load is too dense (observed via HAM records), so energy is wall
    time; f32r matmuls throttled ~38% of the run, bf16 ~10%.
  - Projections pipeline with the input DMAs: the six earliest-needed
    K/Q accumulator groups run d-chunk-outer in PSUM banks borrowed from
    the attention-phase tag rings (a dedicated pool's exit barrier would
    stall the first scores on all eight bias drains), ordered so the two
    tiles the first scores need drain first. The V projection and the
    remaining K/Q groups ride inside the first attention block's j-loop.
  - Attention blocks (pair p, 512 queries) run a 16-step j-loop:
    exp(j) on ACT, scores(j+1) emitted before AV(j) so the PE FIFO never
    queues behind the activation, and the next block's first scores are
    emitted at j=15 so ACT never gaps at block boundaries. softmax
    denominators ride as a 65th row of the AV matmul (v augmented with a
    ones column).
  - Normalization is a DVE reciprocal (deferred into the next block's
    j-loop at j=13, off the fin-cast path) + rank-1 ones-outer-product
    broadcast on the PE + DVE multiply into the bf16 Wo lhsT, all one
    block late so the PE never waits on it.
  - Output projection tiles are spread one-per-j-step into later blocks
    (j>=4, after the block-boundary DVE drain chain clears); outputs stage
    to bf16 SBUF and DMA out per 128-row tile. The host gather upcasts.
  - The final block's drains/casts use the scalar engine (idle at the
    tail and its Copy shares the exp table, so no table swap); elsewhere
    copies stay on DVE because extra concurrent engines raise the power
    duty-cycle throttle more than they help.
"""

import numpy as np

B, S, D, H, DK = 2, 2048, 1024, 16, 64
HPC = 4          # heads per core
HD = HPC * DK    # 256 projected dims per core
P = 128
NB = 512
NCORES = 8

_CACHE = {}


def _install_tile_drain_fix():
    """TileContext._drain_and_barrier piles every outstanding sem wait onto
    one Drain instruction; this walrus build rejects >1 sync wait per
    instruction. Split the extra waits across single-wait NOPs."""
    import concourse.tile as tile
    from concourse.vector_clock import ScopedClock

    if getattr(tile.TileContext, "_ant_drain_fix", False):
        return

    def _drain_and_barrier_split(self, tick_clock, wait_clock):
        drain_inst = self.nc.sync.drain()
        wait_clock.add_sem_waits(
            drain_inst.ins, ScopedClock({None: tick_clock.global_clock})
        )
        waits = list(drain_inst.ins.sync_info.on_wait or [])
        if len(waits) > 1:
            drain_inst.ins.sync_info.on_wait = waits[:1]
            for w in waits[1:]:
                n = self.nc.sync.nop(nofuse=True)
                si = n.ins.sync_info
                if si is None:
                    import bass_rust

                    n.ins.sync_info = bass_rust.SyncInfo(on_wait=[w], on_update=[])
                else:
                    si.on_wait = [w]

        self.nc.all_engine_barrier()
        assert self.sems is not None
        popped = self.nc._tile_sem_poison_stack.pop()
        assert popped is self._sem_poison
        self.nc.clear_and_free_semaphores(list(self.sems.allocated().values()))
        self.nc.all_engine_barrier()

    tile.TileContext._drain_and_barrier = _drain_and_barrier_split
    tile.TileContext._ant_drain_fix = True


def _split_excess_waits(nc):
    """walrus's per-struct sync-wait capacity is small (observed: 1 for the
    self-loading-weight Matmult S3_LW struct, 2 for TPB_CTRL/Drain). Tile's
    wait assignment can leave many waits on one instruction; hoist the excess
    onto NOPs on the same engine immediately before it."""
    import concourse.mybir as mybir

    nid = [0]
    for f in nc.m.functions:
        for bb in f.blocks:
            out = []
            changed = False
            for inst in bb.instructions:
                si = getattr(inst, "sync_info", None)
                waits = list(si.on_wait) if si is not None and si.on_wait else []
                cap = 1
                if len(waits) > cap:
                    extra = waits[cap:]
                    for k in range(0, len(extra), 2):
                        nid[0] += 1
                        out.append(
                            mybir.InstEventSemaphore(
                                name=f"I-waitsplit-{nid[0]}",
                                ins=[],
                                outs=[],
                                sync_info=mybir.SyncInfo(
                                    on_wait=extra[k:k + 2], on_update=[]
                                ),
                                engine=inst.engine,
                            )
                        )
                    si.on_wait = waits[:cap]
                    changed = True
                out.append(inst)
            if changed:
                bb.instructions = out


def _recip_fast(nc, out, in_):
    with nc.allow_low_precision("fp22 recip feeds f32r matmul"):
        nc.vector.reciprocal(out=out, in_=in_)


def _build_program():
    import concourse.bass as bass
    import concourse.mybir as mybir
    from concourse.tile import TileContext

    _install_tile_drain_fix()

    f32 = mybir.dt.float32
    f32r = mybir.dt.float32r
    bf16 = mybir.dt.bfloat16
    Exp = mybir.ActivationFunctionType.Exp

    nc = bass.Bass()

    xt = nc.dram_tensor("xt", [D, S], bf16, kind="ExternalInput")
    # weights arrive pre-packed partition-major from the host, so each DMA
    # moves 4KB-contiguous runs per partition (descriptor-count-bound
    # otherwise: the transposed gather was ~4x slower)
    wq = nc.dram_tensor("wq", [P, (D // P) * HD], bf16, kind="ExternalInput")
    wk = nc.dram_tensor("wk", [P, (D // P) * HD], bf16, kind="ExternalInput")
    wv = nc.dram_tensor("wv", [P, (D // P) * HD], bf16, kind="ExternalInput")
    wo = nc.dram_tensor("wo", [P, 2 * D], bf16, kind="ExternalInput")
    bqt = nc.dram_tensor("bqt", [P, 2], f32, kind="ExternalInput")
    bkt = nc.dram_tensor("bkt", [P, 2], f32, kind="ExternalInput")
    outp = nc.dram_tensor("outp", [S, D], bf16, kind="ExternalOutput")

    NDC = D // P      # 8 d-chunks
    NST = S // P      # 16 sequence tiles
    NSB = S // NB     # 4 sequence blocks

    with TileContext(nc) as tc:
        with tc.tile_pool(name="consts", bufs=1) as consts:
            # memset on a float32r AP emits invalid ISA; write the f32 bit
            # pattern of 1.0 through a uint32 view instead
            onesg = consts.tile([33, DK], bf16)
            nc.vector.memset(onesg.bitcast(mybir.dt.uint16), 0x3F80)

            # DMA queue order follows first-use: biases and K/Q weights
            # (the wave-1 projection needs them first), then xT chunks (the
            # d-outer projection starts as soon as chunk 0 lands), V/O last
            bq_sb = consts.tile([P, 2], f32)
            nc.sync.dma_start(bq_sb[:], bqt[:])
            bk_sb = consts.tile([P, 2], f32)
            nc.sync.dma_start(bk_sb[:], bkt[:])
            wk_sb = consts.tile([P, NDC, HD], bf16)
            nc.sync.dma_start(wk_sb[:], wk.rearrange("p (c h) -> p c h", c=NDC))
            wq_sb = consts.tile([P, NDC, HD], bf16)
            nc.sync.dma_start(wq_sb[:], wq.rearrange("p (c h) -> p c h", c=NDC))
            xT = consts.tile([P, NDC, S], bf16)
            for d in range(NDC):
                nc.sync.dma_start(xT[:, d, :], xt[d * P:(d + 1) * P, :])

            with tc.tile_pool(name="acts", bufs=1) as acts:
                # pair-packed transposed projections: [2 heads x 64, S]
                qT = acts.tile([P, 2, S], bf16)
                kT = acts.tile([P, 2, S], bf16)
                # v augmented with a ones column (row 65 of the AV matmul
                # accumulates the softmax denominator): [s, j-tile, head, 65]
                va = acts.tile([P, NST, HPC, DK + 1], bf16)
                nc.vector.memset(va.bitcast(mybir.dt.uint16), 0x3F80)
                # Wo lhsT: [head-dim pair-chunk, pair, i]
                stack = acts.tile([P, 2, S], bf16)
                # staging tile for both heads' softmax denominators, at
                # partitions 0 and 32 so one reciprocal covers both; the
                # filler rows are preset to 1.0 so recip never sees junk
                wv_sb = acts.tile([P, NDC, HD], bf16)
                nc.sync.dma_start(
                    wv_sb[:], wv.rearrange("p (c h) -> p c h", c=NDC)
                )
                wo_sb = acts.tile([P, 2, D], bf16)
                nc.sync.dma_start(
                    wo_sb[:], wo.rearrange("p (c d) -> p c d", c=2)
                )
                sums_sb = acts.tile([33, NB], bf16)
                nc.vector.memset(sums_sb.bitcast(mybir.dt.uint16), 0x3F80)
                # tiny warm-up exp: hoists the 1.3us ACT table load out of
                # the first real activation's critical path
                warm = acts.tile([1, 2], bf16)
                nc.scalar.activation(warm[:], sums_sb[0:1, 0:2], Exp, scale=0.125)

                # ---- Attention + output projection. V projection rides in
                # block (0,0); Q projections for blocks 1-3 ride in blocks
                # (0,1), (1,0), (2,0). PSUM: sc 4 banks, av0/av1 2, aux 2.
                with (
                    tc.tile_pool(name="ph2", bufs=1) as ph2,
                    tc.tile_pool(name="ph2p", bufs=1, space="PSUM") as ph2p,
                ):
                    # ---- Wave-1 projection: K pair 0 (all blocks), Q block
                    # 0 pair 0, K block 0 pair 1 - six PSUM accumulator
                    # groups borrowed from phase-2's own tag rings (no
                    # separate pool: a pool exit would put an all-drain
                    # barrier in front of the first scores). d-chunk-outer:
                    # layer d only needs xT chunk d, so compute pipelines
                    # with the xT DMAs. The two tiles the first scores need
                    # (k sb0/p0, q b0/p0) drain first, and they sit in the
                    # same sc-ring slot the first scores will request.
                    # dummy accumulation keeps the PE p-state hot through
                    # the initial DMA window so wave-1 streams at full clock
                    warmps = ph2p.tile([P, NB], f32, tag="av0", bufs=1,
                                       name="warmps")
                    for r in range(48):
                        nc.tensor.matmul(
                            warmps[0:DK, 0:DK], onesg[0:33, :], onesg[0:33, :],
                            start=(r == 0), stop=(r == 47),
                        )
                    w1sc0 = ph2p.tile([P, 2 * NB], f32, tag="sc", bufs=2,
                                      name="w1sc0")
                    w1a = ph2p.tile([P, NB], f32, tag="aux", bufs=2,
                                    name="w1a")
                    w1b = ph2p.tile([P, NB], f32, tag="aux", bufs=2,
                                    name="w1b")
                    w1sc1 = ph2p.tile([P, 2 * NB], f32, tag="sc", bufs=2,
                                      name="w1sc1")
                    wave1 = [
                        (w1sc0[:, 0:NB], "k", 0, 0),
                        (w1sc0[:, NB:2 * NB], "q", 0, 0),
                        (w1a[:], "k", 1, 0),
                        (w1b[:], "k", 2, 0),
                        (w1sc1[:, 0:NB], "k", 3, 0),
                        (w1sc1[:, NB:2 * NB], "k", 0, 1),
                    ]
                    for d in range(NDC):
                        for acc, kind, sb_, p_ in wave1:
                            w_sb = wk_sb if kind == "k" else wq_sb
                            nc.tensor.matmul(
                                acc,
                                w_sb[:, d, p_ * P:(p_ + 1) * P],
                                xT[:, d, sb_ * NB:(sb_ + 1) * NB],
                                start=(d == 0),
                                stop=(d == NDC - 1),
                            )
                            if d == NDC - 1:
                                dT, b_sb = (
                                    (kT, bk_sb) if kind == "k" else (qT, bq_sb)
                                )
                                nc.vector.tensor_scalar_add(
                                    out=dT[:, p_, sb_ * NB:(sb_ + 1) * NB],
                                    in0=acc,
                                    scalar1=b_sb[:, p_:p_ + 1],
                                )
                    def emit_vp(jt):
                        vp = ph2p.tile(
                            [P, NB], f32, tag="aux", bufs=2, name=f"vp{jt}"
                        )
                        for d in range(NDC):
                            nc.tensor.matmul(
                                vp[:, 0:HD],
                                xT[:, d, jt * P:(jt + 1) * P],
                                wv_sb[:, d, :],
                                start=(d == 0),
                                stop=(d == NDC - 1),
                            )
                        nc.vector.tensor_copy(
                            out=va[:, jt, :, 0:DK],
                            in_=vp[:, 0:HD].rearrange("p (h e) -> p h e", h=HPC),
                        )

                    def emit_burst(kind, sb_, p_):
                        # one full projection group (8 matmuls + bias drain)
                        # inside block (0,0)'s j-loop: finishes wave-1's
                        # leftovers (k sb3/p1, q b0/p1) without delaying the
                        # first exp
                        ps = ph2p.tile(
                            [P, NB], f32, tag="aux", bufs=2,
                            name=f"burst_{kind}{sb_}_{p_}",
                        )
                        w_sb = wk_sb if kind == "k" else wq_sb
                        for d in range(NDC):
                            nc.tensor.matmul(
                                ps[:],
                                w_sb[:, d, p_ * P:(p_ + 1) * P],
                                xT[:, d, sb_ * NB:(sb_ + 1) * NB],
                                start=(d == 0),
                                stop=(d == NDC - 1),
                            )
                        dT, b_sb = (kT, bk_sb) if kind == "k" else (qT, bq_sb)
                        nc.vector.tensor_scalar_add(
                            out=dT[:, p_, sb_ * NB:(sb_ + 1) * NB],
                            in0=ps[:],
                            scalar1=b_sb[:, p_:p_ + 1],
                        )

                    # qproj for block qb, split in 16 single-matmul steps
                    qstate = {}

                    def emit_qstep(qb, step):
                        p_, d = divmod(step, NDC)
                        if d == 0:
                            qstate[(qb, p_)] = ph2p.tile(
                                [P, NB], f32, tag="aux", bufs=2,
                                name=f"q{qb}_{p_}",
                            )
                        psq = qstate[(qb, p_)]
                        nc.tensor.matmul(
                            psq[:],
                            wq_sb[:, d, p_ * P:(p_ + 1) * P],
                            xT[:, d, qb * NB:(qb + 1) * NB],
                            start=(d == 0),
                            stop=(d == NDC - 1),
                        )
                        if d == NDC - 1:
                            nc.vector.tensor_scalar_add(
                                out=qT[:, p_, qb * NB:(qb + 1) * NB],
                                in0=psq[:],
                                scalar1=bq_sb[:, p_:p_ + 1],
                            )

                    def emit_scores(ib, p, j):
                        sc = ph2p.tile(
                            [P, 2 * NB], f32, tag="sc", bufs=2,
                            name=f"sc{ib}_{p}_{j}",
                        )
                        i0 = ib * NB
                        nc.tensor.matmul(
                            sc[:, 0:NB],
                            kT[0:DK, p, j * P:(j + 1) * P],
                            qT[0:DK, p, i0:i0 + NB],
                            tile_position=(0, 0),
                        )
                        nc.tensor.matmul(
                            sc[:, NB:2 * NB],
                            kT[DK:2 * DK, p, j * P:(j + 1) * P],
                            qT[DK:2 * DK, p, i0:i0 + NB],
                            tile_position=(64, 0),
                        )
                        return sc

                    def make_fin(it, tail=False):
                        # one output row-tile: both 512-halves of the final
                        # projection, staged to bf16 SBUF, then one DMA. The
                        # post-loop fins stage via the scalar engine (idle at
                        # the tail; its copy shares the exp table) so the
                        # drain chain doesn't serialize on DVE
                        def go():
                            ot = ph2.tile(
                                [P, D], bf16, tag="ot", bufs=2, name=f"ot{it}"
                            )
                            for nbi in range(2):
                                ps = ph2p.tile(
                                    [P, NB], f32, tag="aux", bufs=2,
                                    name=f"fin{it}_{nbi}",
                                )
                                for pch in range(2):
                                    nc.tensor.matmul(
                                        ps[:],
                                        stack[:, pch, it * P:(it + 1) * P],
                                        wo_sb[:, pch,
                                              nbi * NB:(nbi + 1) * NB],
                                        start=(pch == 0),
                                        stop=(pch == 1),
                                    )
                                dst = ot[:, nbi * NB:(nbi + 1) * NB]
                                if tail and nbi == 1:
                                    nc.scalar.copy(out=dst, in_=ps[:])
                                else:
                                    nc.vector.tensor_copy(out=dst, in_=ps[:])
                            nc.sync.dma_start(outp[it * P:(it + 1) * P, :], ot[:])
                        return go

                    def finish_norm(ib, p, po_sbs, rc33, tail=False):
                        # broadcast each head's 1/sumexp across 64 partitions
                        # (rank-1 matmul) and scale the raw AV numerators into
                        # the Wo lhsT. Emitted one block late so the PE never
                        # waits on the DVE reciprocals. Returns the final
                        # projection closures (spread into a later j-loop).
                        i0 = ib * NB
                        for hh in range(2):
                            bc = ph2p.tile(
                                [P, NB], f32, tag="aux", bufs=2,
                                name=f"bc{ib}_{p}_{hh}",
                            )
                            nc.tensor.matmul(
                                bc[0:DK, :],
                                onesg[32 * hh:32 * hh + 1, :],
                                rc33[32 * hh:32 * hh + 1, :],
                                tile_position=(32 * hh, 0),
                            )
                            nc.vector.tensor_tensor(
                                out=stack[hh * DK:(hh + 1) * DK, p, i0:i0 + NB],
                                in0=po_sbs[hh][0:DK, :],
                                in1=bc[0:DK, :],
                                op=mybir.AluOpType.mult,
                            )
                        if p != 1:
                            return []
                        return [
                            make_fin(ib * (NB // P) + t, tail)
                            for t in range(NB // P)
                        ]

                    # extra projection work carried by each block's j-loop;
                    # fin batches ride two blocks after their AV block, in
                    # blocks with no projection job (aux-ring discipline:
                    # a held qproj slot must never interleave with fins)
                    carry = {(0, 0): "v", (0, 1): 1, (1, 0): 2, (2, 0): 3}
                    blocks = [(ib, p) for ib in range(NSB) for p in range(2)]

                    pending_norm = None
                    pending_fins = []
                    pending_recip = None
                    sc = None
                    for bi, (ib, p) in enumerate(blocks):
                        job = carry.get((ib, p))
                        last = bi == len(blocks) - 1
                        po0 = ph2p.tile(
                            [P, NB], f32, tag="av0", bufs=1,
                            name=f"po0_{ib}_{p}",
                        )
                        po1 = ph2p.tile(
                            [P, NB], f32, tag="av1", bufs=1,
                            name=f"po1_{ib}_{p}",
                        )
                        if sc is None:
                            sc = emit_scores(ib, p, 0)
                        if job == "v":
                            emit_vp(0)
                        fin_q = pending_fins
                        pending_fins = []
                        for j in range(NST):
                            ex = ph2.tile(
                                [P, 2 * NB], bf16, tag="ex", bufs=5,
                                name=f"ex{ib}_{p}_{j}",
                            )
                            nc.scalar.activation(
                                ex[:], sc[:], Exp, scale=0.125
                            )
                            # scores for the next step (or the next block's
                            # first step) are emitted before AV(j) so the PE
                            # FIFO never queues behind exp(j) and the ACT
                            # engine never gaps at block boundaries
                            if j + 1 < NST:
                                sc = emit_scores(ib, p, j + 1)
                            elif bi + 1 < len(blocks):
                                sc = emit_scores(*blocks[bi + 1], 0)
                            if job == "v":
                                # block (0,0) also finishes the projection
                                # groups wave-1 had no PSUM room for, as
                                # j-step bursts
                                if j < 4:
                                    emit_burst(*(
                                        ("k", 1, 1), ("q", 0, 1),
                                        ("k", 2, 1), ("k", 3, 1),
                                    )[j])
                                if j + 1 < NST:
                                    emit_vp(j + 1)
                            elif job is not None:
                                emit_qstep(job, j)
                            elif fin_q and j >= 4:
                                # j>=4: the block-boundary DVE chain (po
                                # drains, sums, norms) must clear before fin
                                # casts queue up, or fin matmuls block AV
                                fin_q.pop(0)()
                            # the previous block's reciprocal runs mid-loop so
                            # it never delays this block's fin casts on DVE
                            if pending_recip is not None and j == (
                                2 if last else 13
                            ):
                                pending_recip()
                                pending_recip = None
                            if last and pending_norm is not None and j == 14:
                                # the (3,0) normalize runs inside this loop so
                                # the tail's serial DVE chain starts at the
                                # (3,1) reciprocal, not two norms earlier
                                pending_fins = finish_norm(*pending_norm)
                                pending_norm = None
                            nc.tensor.matmul(
                                po0[0:DK + 1, :],
                                va[:, j, 2 * p, :],
                                ex[:, 0:NB],
                                start=(j == 0),
                                stop=(j == NST - 1),
                            )
                            nc.tensor.matmul(
                                po1[0:DK + 1, :],
                                va[:, j, 2 * p + 1, :],
                                ex[:, NB:2 * NB],
                                start=(j == 0),
                                stop=(j == NST - 1),
                            )
                        assert not fin_q, (ib, p, len(fin_q))
                        # drain both accumulator banks so the next block's AV
                        # can start (all on DVE: the core power-throttles when
                        # too many engines run dense, so spreading copies onto
                        # ACT mid-stream backfires)
                        po_sbs = []
                        for hh, po in ((0, po0), (1, po1)):
                            po_sb = ph2.tile(
                                [DK, NB], bf16, tag="posb", bufs=4,
                                name=f"posb{ib}_{p}_{hh}",
                            )
                            if last:
                                # tail: scalar engine (idle there) drains the
                                # accumulators while DVE runs the reciprocal
                                nc.scalar.copy(out=po_sb[:], in_=po[0:DK, :])
                            else:
                                nc.vector.tensor_copy(
                                    out=po_sb[:], in_=po[0:DK, :]
                                )
                            po_sbs.append(po_sb)
                        # partition-shifting copies (64 -> 0/32): proven
                        # on DVE, keep them there
                        nc.vector.tensor_copy(
                            out=sums_sb[0:1, :], in_=po0[DK:DK + 1, :]
                        )
                        nc.vector.tensor_copy(
                            out=sums_sb[32:33, :], in_=po1[DK:DK + 1, :]
                        )
                        rc33 = ph2.tile(
                            [33, NB], bf16, tag="rc", bufs=2,
                            name=f"rc{ib}_{p}",
                        )
                        if last:
                            _recip_fast(nc, rc33[:], sums_sb[:])
                        else:
                            def _defer(rc=rc33):
                                _recip_fast(nc, rc[:], sums_sb[:])
                            pending_recip = _defer
                        # norms for the previous block go here (not at block
                        # start): their bc matmuls consume a reciprocal that
                        # by now is a full block old, so the PE never waits
                        if pending_norm is not None:
                            pending_fins = finish_norm(*pending_norm)
                        pending_norm = (ib, p, po_sbs, rc33)
                    for go in pending_fins + finish_norm(*pending_norm, tail=True):
                        go()

    _split_excess_waits(nc)
    return nc


def _get_program():
    if "nc" not in _CACHE:
        _CACHE["nc"] = _build_program()
    return _CACHE["nc"]


def kernel(x, Wq, bq, Wk, bk, Wv, bv, Wo, bo, _trace=False):
    import ml_dtypes
    from concourse.bass_utils import run_bass_kernel_spmd

    bf16 = ml_dtypes.bfloat16
    x = np.asarray(x, dtype=np.float32)
    Wq = np.asarray(Wq, dtype=np.float32)
    Wk = np.asarray(Wk, dtype=np.float32)
    Wv = np.asarray(Wv, dtype=np.float32)
    Wo = np.asarray(Wo, dtype=np.float32)
    bq = np.asarray(bq, dtype=np.float32)
    bk = np.asarray(bk, dtype=np.float32)
    bv = np.asarray(bv, dtype=np.float32)
    bo = np.asarray(bo, dtype=np.float32)

    def pack(w):
        # [NDC*P, F] -> partition-major [P, NDC*F] so SBUF DMA runs are
        # contiguous per partition
        ndc = w.shape[0] // P
        return np.ascontiguousarray(
            w.reshape(ndc, P, -1).transpose(1, 0, 2).reshape(P, -1)
        ).astype(bf16)

    xtb = [np.ascontiguousarray(x[b].T).astype(bf16) for b in range(B)]
    in_maps = []
    for c in range(NCORES):
        b = c // 4
        cs = (c % 4) * HD
        in_maps.append({
            "xt": xtb[b],
            "wq": pack(Wq[:, cs:cs + HD]),
            "wk": pack(Wk[:, cs:cs + HD]),
            "wv": pack(Wv[:, cs:cs + HD]),
            "wo": pack(Wo[cs:cs + HD, :]),
            "bqt": np.ascontiguousarray(bq[cs:cs + HD].reshape(2, P).T),
            "bkt": np.ascontiguousarray(bk[cs:cs + HD].reshape(2, P).T),
        })

    nc = _get_program()
    res = run_bass_kernel_spmd(
        nc, in_maps, core_ids=list(range(NCORES)), trace=_trace
    )

    cvec = (bv @ Wo + bo).astype(np.float32)
    out = np.empty((B, S, D), dtype=np.float32)
    for b in range(B):
        acc = res.results[4 * b]["outp"].astype(np.float64)
        for c in range(4 * b + 1, 4 * b + 4):
            acc = acc + res.results[c]["outp"]
        out[b] = (acc + cvec).astype(np.float32)

    if _trace:
        _CACHE["last_results"] = res
    return out
